# revision 21
# baseline (speedup 1.0000x reference)
"""Causal self-attention Trainium2 kernel (8 NeuronCores, SPMD, no collectives).

Problem (from reference):
    B=4, T=2048, C=1024, NH=16, HS=64
    qkv = x @ w_attn + b_attn ; heads interleaved: head h uses cols d*NH+h
    att[i,j] = q_i.k_j/sqrt(HS), keep i<=j, softmax over i
    y[d,j] = sum_i v[i,d] p[i,j] ; out = y @ w_proj + b_proj

Sharding: core = 2*b + g  (b in 0..3 batch, g in 0..1 head-group of 8 heads).
Each core computes a partial output (its 8 heads' contribution to out[b]);
host sums the two partials per batch and adds biases.

Device pipeline per core (all bf16 matmuls, fp32 PSUM):
    xT[b] (host pre-transposed, bf16)  --MM-->  qT,kT (channel-major) and
    v (token-major, via lhsT=xT), then per head flash-style causal attention
    with softmax denominator via a ones-column in the V stationary operand,
    then y^T @ w_proj_rows.

Host-side exact bias handling: q-bias drops out of softmax_i (constant in i);
k-bias added on device per-partition; v-bias and b_proj folded in on host.
"""

import os
import sys

import numpy as np

if "/opt/trn_rl_repo" not in sys.path:
    sys.path.insert(0, "/opt/trn_rl_repo")

import ml_dtypes

BF16 = ml_dtypes.bfloat16

B, T, C, NH, HS = 4, 2048, 1024, 16, 64
HPC = 8            # heads per core
DH = HPC * HS      # 512 head-channels per core
P = 128
NCORES = 8

_NC_CACHE = {}


def build_bass(t=T, c=C, hpc=HPC, hs=HS, debug_taps=False):
    """Build + compile the per-core Bass program. Parameterized so small
    versions can be simulated for debugging."""
    import concourse.bass as bass  # noqa: F401
    import concourse.tile as tile
    from concourse import bacc, mybir

    dt = mybir.dt
    f32 = dt.float32
    bf16 = dt.bfloat16
    Exp = mybir.ActivationFunctionType.Exp
    add_op = mybir.AluOpType.add

    dh = hpc * hs
    KC = c // P            # contraction chunks over C
    NT = t // P            # token tiles
    QCH = dh // P          # channel chunks for q (and for k) per core
    NB = t // 512          # 512-wide t blocks
    JB = 512               # j block width

    nc = bacc.Bacc("TRN2", target_bir_lowering=False, debug=False)

    xt_d = nc.dram_tensor("xt", [c, t], bf16, kind="ExternalInput").ap()
    wqk_d = nc.dram_tensor("wqk", [c, 2 * dh], bf16, kind="ExternalInput").ap()
    wv_d = nc.dram_tensor("wv", [c, dh], bf16, kind="ExternalInput").ap()
    wp_d = nc.dram_tensor("wp", [dh, c], bf16, kind="ExternalInput").ap()
    bk_d = nc.dram_tensor("bk", [dh], f32, kind="ExternalInput").ap()
    masks_d = nc.dram_tensor("masks", [P, 4, JB], bf16, kind="ExternalInput").ap()
    out_d = nc.dram_tensor("out", [t, c], f32, kind="ExternalOutput").ap()
    dbg = {}
    if debug_taps:
        dbg["q"] = nc.dram_tensor("dbg_q", [P, hpc, t], bf16, kind="ExternalOutput").ap()
        dbg["k"] = nc.dram_tensor("dbg_k", [P, (hpc * hs) // P, t], bf16, kind="ExternalOutput").ap()
        dbg["v"] = nc.dram_tensor("dbg_v", [P, t // P, hpc, P], bf16, kind="ExternalOutput").ap()
        dbg["y"] = nc.dram_tensor("dbg_y", [P, (hpc * hs) // P, t], bf16, kind="ExternalOutput").ap()
        dbg["lb"] = nc.dram_tensor("dbg_lb", [hpc, P, t], f32, kind="ExternalOutput").ap()
        dbg["yps"] = nc.dram_tensor("dbg_yps", [P, t], f32, kind="ExternalOutput").ap()
        dbg["s0"] = nc.dram_tensor("dbg_s0", [2, P, 1024], f32, kind="ExternalOutput").ap()
        dbg["p0"] = nc.dram_tensor("dbg_p0", [2, P, 1024], bf16, kind="ExternalOutput").ap()

    with tile.TileContext(nc) as tc:
        from contextlib import ExitStack

        with ExitStack() as ctx:
            const = ctx.enter_context(tc.tile_pool(name="const", bufs=1))
            data = ctx.enter_context(tc.tile_pool(name="data", bufs=1))
            # phase-A-only tensors; pool released before attention phase
            adata_cm = tc.tile_pool(name="adata", bufs=1)
            adata = adata_cm.__enter__()

            # ---------------- input loads ----------------
            xt_sb = adata.tile([P, KC, t], bf16)
            xt_r = xt_d.rearrange("(o p) t -> p o t", p=P)
            for kc in range(KC):
                nc.sync.dma_start(xt_sb[:, kc], xt_r[:, kc])

            wqk_sb = adata.tile([P, KC, 2 * dh], bf16)
            wqk_r = wqk_d.rearrange("(o p) n -> p o n", p=P)
            for kc in range(KC):
                nc.sync.dma_start(wqk_sb[:, kc], wqk_r[:, kc])

            wv_sb = adata.tile([P, KC, dh], bf16)
            wv_r = wv_d.rearrange("(o p) n -> p o n", p=P)
            for kc in range(KC):
                nc.sync.dma_start(wv_sb[:, kc], wv_r[:, kc])

            wp_sb = data.tile([P, dh // P, c], bf16)
            wp_r = wp_d.rearrange("(o p) n -> p o n", p=P)
            for kc in range(dh // P):
                nc.sync.dma_start(wp_sb[:, kc], wp_r[:, kc])

            bk_sb = const.tile([P, dh // P], f32)
            nc.sync.dma_start(bk_sb[:], bk_d.rearrange("(o p) -> p o", p=P))

            mask_sb = const.tile([P, 4, JB], bf16)
            nc.sync.dma_start(mask_sb[:], masks_d)

            # persistent activations
            # q is stored zero-padded per head: chunk h holds head h's 64
            # channels at partitions pr..pr+63 (pr = (h%2)*64), zeros
            # elsewhere, so S matmuls contract over the full 128 partitions
            # (K<128 matmuls are unreliable on HW).
            q_sb = data.tile([P, hpc, t], bf16)
            k_sb = data.tile([P, QCH, t], bf16)
            nc.vector.memset(q_sb[:], 0.0)
            # V stationary operand: per (token-tile, head) a [P,128] block:
            #   even head: cols 0..63 = v, col 64 = 1.0  -> y at part 0..63, l at 64
            #   odd  head: col 32 = 1.0, cols 64..127 = v -> l at 32, y at part 64..127
            v_sb = data.tile([P, NT, hpc, P], bf16)
            y_sb = data.tile([P, dh // P, t], bf16)

            nc.vector.memset(v_sb[:], 0.0)
            nc.vector.memset(v_sb[:, :, 0::2, 64], 1.0)
            nc.vector.memset(v_sb[:, :, 1::2, 32], 1.0)

            # ---------------- phase A: projections ----------------
            with tc.tile_pool(name="psA", bufs=4, space="PSUM") as psA:
                # q/k channel-major: qkT[cc, t] = sum_c' Wqk[c',cc] * xT[c',t]
                for m in range(2 * QCH):
                    dst = q_sb if m < QCH else k_sb
                    mm = m % QCH
                    for nb in range(NB):
                        ps = psA.tile([P, 512], f32, tag="psA", name="psA")
                        for kc in range(KC):
                            nc.tensor.matmul(
                                ps[:],
                                wqk_sb[:, kc, m * P:(m + 1) * P],
                                xt_sb[:, kc, nb * 512:(nb + 1) * 512],
                                start=(kc == 0),
                                stop=(kc == KC - 1),
                            )
                        if m >= QCH:
                            # fold in k-bias (per output channel == partition)
                            nc.vector.tensor_scalar(
                                dst[:, mm, nb * 512:(nb + 1) * 512],
                                ps[:],
                                bk_sb[:, mm:mm + 1],
                                None,
                                add_op,
                            )
                        else:
                            sl = slice(nb * 512, (nb + 1) * 512)
                            nc.vector.tensor_copy(
                                q_sb[0:64, 2 * mm, sl], ps[0:64]
                            )
                            nc.vector.tensor_copy(
                                q_sb[64:128, 2 * mm + 1, sl], ps[64:128]
                            )
                # v token-major: v[t, dv] = sum_c' xT[c',t] * Wv[c',dv]
                for tt in range(NT):
                    ps = psA.tile([P, dh], f32, tag="psA", name="psA")
                    for kc in range(KC):
                        nc.tensor.matmul(
                            ps[:],
                            xt_sb[:, kc, tt * P:(tt + 1) * P],
                            wv_sb[:, kc, :],
                            start=(kc == 0),
                            stop=(kc == KC - 1),
                        )
                    psh = ps.rearrange("p (h d) -> p h d", d=hs)
                    nc.vector.tensor_copy(v_sb[:, tt, 0::2, 0:hs], psh[:, 0::2])
                    nc.vector.tensor_copy(v_sb[:, tt, 1::2, hs:2 * hs], psh[:, 1::2])

            adata_cm.__exit__(None, None, None)

            # ---------------- phase B: attention ----------------
            with tc.tile_pool(name="psY", bufs=1, space="PSUM") as psY, \
                 tc.tile_pool(name="psS", bufs=2, space="PSUM") as psS, \
                 tc.tile_pool(name="pP", bufs=3) as pP, \
                 tc.tile_pool(name="ldram", bufs=2, space="DRAM") as ldram, \
                 tc.tile_pool(name="ytail", bufs=1 if debug_taps else 2) as ytail:
                n_jb = t // JB
                for h in range(hpc):
                    ch = h // 2
                    pr = (h % 2) * 64
                    lrow = 64 if h % 2 == 0 else 32
                    y_ps = psY.tile([P, t], f32, tag="y", name="y_ps")
                    for ic in range(NT):
                        i0 = ic * P
                        jb0 = i0 // JB
                        for pstart in range(jb0, n_jb, 2):
                            pb = min(2, n_jb - pstart)
                            w = pb * JB
                            s_ps = psS.tile([P, 2 * JB], f32, tag="s", name="s_ps")
                            for bb in range(pb):
                                jb = pstart + bb
                                nc.tensor.matmul(
                                    s_ps[:, bb * JB:(bb + 1) * JB],
                                    q_sb[:, h, i0:i0 + P],
                                    k_sb[:, ch, jb * JB:(jb + 1) * JB],
                                    start=True,
                                    stop=True,
                                )
                            p_t = pP.tile([P, 2 * JB], bf16, tag="p", name="p_t")
                            nc.scalar.activation(
                                p_t[:, :w], s_ps[:, :w], Exp,
                                scale=float(1.0 / np.sqrt(hs)),
                            )
                            if pstart == jb0:
                                m4 = ic % (JB // P)
                                nc.vector.tensor_mul(
                                    p_t[:, :JB], p_t[:, :JB], mask_sb[:, m4]
                                )
                            if debug_taps and h == 0 and ic in (0, 5) and pstart == (ic * P) // JB:
                                di = 0 if ic == 0 else 1
                                s_tap = pP.tile([P, 2 * JB], f32, tag="s_tap", name="s_tap")
                                nc.vector.tensor_copy(s_tap[:, :w], s_ps[:, :w])
                                nc.sync.dma_start(dbg["s0"][di, :, :w], s_tap[:, :w])
                                nc.sync.dma_start(dbg["p0"][di, :, :w], p_t[:, :w])
                            for bb in range(pb):
                                jb = pstart + bb
                                nc.tensor.matmul(
                                    y_ps[:, jb * JB:(jb + 1) * JB],
                                    v_sb[:, ic, h, :],
                                    p_t[:, bb * JB:(bb + 1) * JB],
                                    start=(ic == 0),
                                    stop=(ic == (jb + 1) * (JB // P) - 1),
                                    skip_group_check=True,
                                )
                    # normalize: y / l  (l = ones-row accumulation)
                    if debug_taps and h == 0:
                        y_tap = ytail.tile([P, t], f32, tag="y_tap", name="y_tap")
                        nc.vector.tensor_copy(y_tap[:], y_ps[:])
                        nc.sync.dma_start(dbg["yps"][:], y_tap[:])
                    # l row: psum -> sbuf (same partition), bounce through DRAM
                    # to broadcast across partitions, then reciprocal at base 0.
                    lrow_sb = ytail.tile([P, t], f32, tag="lrow", name="lrow_sb")
                    nc.vector.tensor_copy(
                        lrow_sb[lrow:lrow + 1, :], y_ps[lrow:lrow + 1, :]
                    )
                    l_dram = ldram.tile([t], f32, tag="ldram", name="l_dram")
                    nc.sync.dma_start(l_dram[:], lrow_sb[lrow:lrow + 1, :])
                    lraw = ytail.tile([P, t], f32, tag="lraw", name="lraw")
                    nc.sync.dma_start(lraw[:], l_dram.partition_broadcast(P))
                    lb = ytail.tile([P, t], f32, tag="lb", name="lb")
                    nc.vector.reciprocal(lb[:], lraw[:])
                    if debug_taps:
                        nc.sync.dma_start(dbg["lb"][h], lb[:])
                    for jb in range(n_jb):
                        sl = slice(jb * JB, (jb + 1) * JB)
                        nc.vector.tensor_mul(
                            y_sb[pr:pr + 64, ch, sl],
                            y_ps[pr:pr + 64, sl],
                            lb[pr:pr + 64, sl],
                        )

            if debug_taps:
                nc.sync.dma_start(dbg["q"][:], q_sb[:])
                nc.sync.dma_start(dbg["k"][:], k_sb[:])
                nc.sync.dma_start(dbg["v"][:], v_sb[:])
                nc.sync.dma_start(dbg["y"][:], y_sb[:])

            # ---------------- phase C: output projection ----------------
            with tc.tile_pool(name="psC", bufs=4, space="PSUM") as psC, \
                 tc.tile_pool(name="oC", bufs=3) as oC:
                for tt in range(NT):
                    for nh in range(c // 512):
                        ps = psC.tile([P, 512], f32, tag="c", name="psC")
                        for kc in range(dh // P):
                            nc.tensor.matmul(
                                ps[:],
                                y_sb[:, kc, tt * P:(tt + 1) * P],
                                wp_sb[:, kc, nh * 512:(nh + 1) * 512],
                                start=(kc == 0),
                                stop=(kc == dh // P - 1),
                            )
                        o_t = oC.tile([P, 512], f32, tag="o", name="o_t")
                        nc.vector.tensor_copy(o_t[:], ps[:])
                        nc.sync.dma_start(
                            out_d[tt * P:(tt + 1) * P, nh * 512:(nh + 1) * 512],
                            o_t[:],
                        )

    nc.compile()
    return nc


def _get_nc():
    if "nc" not in _NC_CACHE:
        _NC_CACHE["nc"] = build_bass()
    return _NC_CACHE["nc"]


def _head_perm(g, nh=NH, hs=HS, hpc=HPC):
    """Column indices of w_attn's q block for head-group g, ordered (h_local, d)."""
    return np.array(
        [d * nh + (hpc * g + h) for h in range(hpc) for d in range(hs)],
        dtype=np.int64,
    )


def _make_masks():
    """masks[i, m, j] = 1.0 if j >= m*128 + i else 0 ; [128, 4, 512] bf16."""
    i = np.arange(P)[:, None, None]
    m = np.arange(4)[None, :, None]
    j = np.arange(512)[None, None, :]
    return (j >= m * P + i).astype(BF16)


def prepare_in_maps(x, w_attn, b_attn, w_proj):
    """Host-side sharding: returns (in_maps list of 8 dicts, bias correction [C])."""
    x = np.asarray(x, dtype=np.float32)
    w_attn = np.asarray(w_attn, dtype=np.float32)
    b_attn = np.asarray(b_attn, dtype=np.float32)
    w_proj = np.asarray(w_proj, dtype=np.float32)

    masks = _make_masks()
    per_g = {}
    bias_corr = np.zeros((C,), dtype=np.float32)
    for g in (0, 1):
        idx = _head_perm(g)
        wqk = np.concatenate(
            [w_attn[:, idx], w_attn[:, C + idx]], axis=1
        ).astype(BF16)
        wv = np.ascontiguousarray(w_attn[:, 2 * C + idx]).astype(BF16)
        wp = np.ascontiguousarray(w_proj[idx, :]).astype(BF16)
        bk = np.ascontiguousarray(b_attn[C + idx]).astype(np.float32)
        bv = b_attn[2 * C + idx]
        bias_corr += bv.astype(np.float32) @ w_proj[idx, :]
        per_g[g] = (wqk, wv, wp, bk)

    in_maps = []
    for core in range(NCORES):
        b, g = core // 2, core % 2
        wqk, wv, wp, bk = per_g[g]
        in_maps.append(
            {
                "xt": np.ascontiguousarray(x[b].T).astype(BF16),
                "wqk": wqk,
                "wv": wv,
                "wp": wp,
                "bk": bk,
                "masks": masks,
            }
        )
    return in_maps, bias_corr


def combine_outputs(outs, b_proj, bias_corr):
    b_proj = np.asarray(b_proj, dtype=np.float32)
    y = np.empty((B, T, C), dtype=np.float32)
    for b in range(B):
        y[b] = outs[2 * b] + outs[2 * b + 1]
    y += (b_proj + bias_corr)[None, None, :]
    return y


def kernel(x=None, w_attn=None, b_attn=None, w_proj=None, b_proj=None, **_):
    from concourse.bass_utils import run_bass_kernel_spmd

    nc = _get_nc()
    in_maps, bias_corr = prepare_in_maps(x, w_attn, b_attn, w_proj)
    res = run_bass_kernel_spmd(nc, in_maps, core_ids=list(range(NCORES)))
    outs = [np.asarray(res.results[c]["out"], dtype=np.float32) for c in range(NCORES)]
    return combine_outputs(outs, b_proj, bias_corr)


# revision 23
# speedup vs baseline: 1.5645x; 1.5645x over previous
"""Causal self-attention Trainium2 kernel (8 NeuronCores, SPMD, no collectives).

Problem (from reference):
    B=4, T=2048, C=1024, NH=16, HS=64
    qkv = x @ w_attn + b_attn ; heads interleaved: head h uses cols d*NH+h
    att[i,j] = q_i.k_j/sqrt(HS), keep i<=j, softmax over i
    y[d,j] = sum_i v[i,d] p[i,j] ; out = y @ w_proj + b_proj

Sharding: core = 2*b + g  (b in 0..3 batch, g in 0..1 head-group of 8 heads).
Each core computes a partial output (its 8 heads' contribution to out[b]);
host sums the two partials per batch and adds biases.

Device pipeline per core (all bf16 matmuls, fp32 PSUM):
    xT[b] (host pre-transposed, bf16)  --MM-->  qT,kT (channel-major) and
    v (token-major, via lhsT=xT), then per head flash-style causal attention
    with softmax denominator via a ones-column in the V stationary operand,
    then y^T @ w_proj_rows.

Host-side exact bias handling: q-bias drops out of softmax_i (constant in i);
k-bias added on device per-partition; v-bias and b_proj folded in on host.
"""

import os
import sys

import numpy as np

if "/opt/trn_rl_repo" not in sys.path:
    sys.path.insert(0, "/opt/trn_rl_repo")

import ml_dtypes

BF16 = ml_dtypes.bfloat16

B, T, C, NH, HS = 4, 2048, 1024, 16, 64
HPC = 8            # heads per core
DH = HPC * HS      # 512 head-channels per core
P = 128
NCORES = 8

_NC_CACHE = {}


def build_bass(t=T, c=C, hpc=HPC, hs=HS, debug_taps=False):
    """Build + compile the per-core Bass program. Parameterized so small
    versions can be simulated for debugging."""
    import concourse.bass as bass  # noqa: F401
    import concourse.tile as tile
    from concourse import bacc, mybir

    dt = mybir.dt
    f32 = dt.float32
    bf16 = dt.bfloat16
    Exp = mybir.ActivationFunctionType.Exp
    add_op = mybir.AluOpType.add

    dh = hpc * hs
    KC = c // P            # contraction chunks over C
    NT = t // P            # token tiles
    QCH = dh // P          # channel chunks for q (and for k) per core
    NB = t // 512          # 512-wide t blocks
    JB = 512               # j block width

    nc = bacc.Bacc("TRN2", target_bir_lowering=False, debug=False)

    xt_d = nc.dram_tensor("xt", [c, t], bf16, kind="ExternalInput").ap()
    wqk_d = nc.dram_tensor("wqk", [c, 2 * dh], bf16, kind="ExternalInput").ap()
    wv_d = nc.dram_tensor("wv", [c, dh], bf16, kind="ExternalInput").ap()
    wp_d = nc.dram_tensor("wp", [dh, c], bf16, kind="ExternalInput").ap()
    bk_d = nc.dram_tensor("bk", [dh], f32, kind="ExternalInput").ap()
    masks_d = nc.dram_tensor("masks", [P, 4, JB], bf16, kind="ExternalInput").ap()
    out_d = nc.dram_tensor("out", [t, c], f32, kind="ExternalOutput").ap()
    dbg = {}
    if debug_taps:
        dbg["q"] = nc.dram_tensor("dbg_q", [P, hpc, t], bf16, kind="ExternalOutput").ap()
        dbg["k"] = nc.dram_tensor("dbg_k", [P, (hpc * hs) // P, t], bf16, kind="ExternalOutput").ap()
        dbg["v"] = nc.dram_tensor("dbg_v", [P, t // P, hpc, P], bf16, kind="ExternalOutput").ap()
        dbg["y"] = nc.dram_tensor("dbg_y", [P, (hpc * hs) // P, t], bf16, kind="ExternalOutput").ap()
        dbg["lb"] = nc.dram_tensor("dbg_lb", [hpc, P, t], f32, kind="ExternalOutput").ap()
        dbg["yps"] = nc.dram_tensor("dbg_yps", [P, t], f32, kind="ExternalOutput").ap()
        dbg["s0"] = nc.dram_tensor("dbg_s0", [2, P, 1024], f32, kind="ExternalOutput").ap()
        dbg["p0"] = nc.dram_tensor("dbg_p0", [2, P, 1024], bf16, kind="ExternalOutput").ap()

    with tile.TileContext(nc) as tc:
        from contextlib import ExitStack

        with ExitStack() as ctx:
            const = ctx.enter_context(tc.tile_pool(name="const", bufs=1))
            data = ctx.enter_context(tc.tile_pool(name="data", bufs=1))
            # phase-A-only tensors; pool released before attention phase
            adata_cm = tc.tile_pool(name="adata", bufs=1)
            adata = adata_cm.__enter__()

            # ---------------- input loads ----------------
            xt_sb = adata.tile([P, KC, t], bf16)
            xt_r = xt_d.rearrange("(o p) t -> p o t", p=P)
            for kc in range(KC):
                nc.sync.dma_start(xt_sb[:, kc], xt_r[:, kc])

            wqk_sb = adata.tile([P, KC, 2 * dh], bf16)
            wqk_r = wqk_d.rearrange("(o p) n -> p o n", p=P)
            for kc in range(KC):
                nc.sync.dma_start(wqk_sb[:, kc], wqk_r[:, kc])

            wv_sb = adata.tile([P, KC, dh], bf16)
            wv_r = wv_d.rearrange("(o p) n -> p o n", p=P)
            for kc in range(KC):
                nc.sync.dma_start(wv_sb[:, kc], wv_r[:, kc])

            wp_sb = data.tile([P, dh // P, c], bf16)
            wp_r = wp_d.rearrange("(o p) n -> p o n", p=P)
            for kc in range(dh // P):
                nc.sync.dma_start(wp_sb[:, kc], wp_r[:, kc])

            bk_sb = const.tile([P, dh // P], f32)
            nc.sync.dma_start(bk_sb[:], bk_d.rearrange("(o p) -> p o", p=P))

            mask_sb = const.tile([P, 4, JB], bf16)
            nc.sync.dma_start(mask_sb[:], masks_d)

            # persistent activations
            # q is stored zero-padded per head: chunk h holds head h's 64
            # channels at partitions pr..pr+63 (pr = (h%2)*64), zeros
            # elsewhere, so S matmuls contract over the full 128 partitions
            # (K<128 matmuls are unreliable on HW).
            q_sb = data.tile([P, hpc, t], bf16)
            k_sb = data.tile([P, QCH, t], bf16)
            nc.gpsimd.memset(q_sb[:], 0.0)
            # V stationary operand: per (token-tile, head) a [P,128] block:
            #   even head: cols 0..63 = v, col 64 = 1.0  -> y at part 0..63, l at 64
            #   odd  head: col 32 = 1.0, cols 64..127 = v -> l at 32, y at part 64..127
            v_sb = data.tile([P, NT, hpc, P], bf16)
            y_sb = data.tile([P, dh // P, t], bf16)

            nc.gpsimd.memset(v_sb[:], 0.0)
            nc.vector.memset(v_sb[:, :, 0::2, 64], 1.0)
            nc.vector.memset(v_sb[:, :, 1::2, 32], 1.0)

            # ---------------- phase A: projections ----------------
            with tc.tile_pool(name="psA", bufs=4, space="PSUM") as psA:
                # q/k channel-major: qkT[cc, t] = sum_c' Wqk[c',cc] * xT[c',t]
                for m in range(2 * QCH):
                    dst = q_sb if m < QCH else k_sb
                    mm = m % QCH
                    for nb in range(NB):
                        ps = psA.tile([P, 512], f32, tag="psA", name="psA")
                        for kc in range(KC):
                            nc.tensor.matmul(
                                ps[:],
                                wqk_sb[:, kc, m * P:(m + 1) * P],
                                xt_sb[:, kc, nb * 512:(nb + 1) * 512],
                                start=(kc == 0),
                                stop=(kc == KC - 1),
                            )
                        if m >= QCH:
                            # fold in k-bias (per output channel == partition)
                            nc.vector.tensor_scalar(
                                dst[:, mm, nb * 512:(nb + 1) * 512],
                                ps[:],
                                bk_sb[:, mm:mm + 1],
                                None,
                                add_op,
                            )
                        else:
                            sl = slice(nb * 512, (nb + 1) * 512)
                            nc.vector.tensor_copy(
                                q_sb[0:64, 2 * mm, sl], ps[0:64]
                            )
                            nc.vector.tensor_copy(
                                q_sb[64:128, 2 * mm + 1, sl], ps[64:128]
                            )
                # v token-major: v[t, dv] = sum_c' xT[c',t] * Wv[c',dv]
                for tt in range(NT):
                    ps = psA.tile([P, dh], f32, tag="psA", name="psA")
                    for kc in range(KC):
                        nc.tensor.matmul(
                            ps[:],
                            xt_sb[:, kc, tt * P:(tt + 1) * P],
                            wv_sb[:, kc, :],
                            start=(kc == 0),
                            stop=(kc == KC - 1),
                        )
                    psh = ps.rearrange("p (h d) -> p h d", d=hs)
                    nc.vector.tensor_copy(v_sb[:, tt, 0::2, 0:hs], psh[:, 0::2])
                    nc.vector.tensor_copy(v_sb[:, tt, 1::2, hs:2 * hs], psh[:, 1::2])

            adata_cm.__exit__(None, None, None)

            # ---------------- phase B: attention ----------------
            # processed per (head, j-slab of 1024): y PSUM double-buffered so
            # the per-slab softmax-denominator tail overlaps the next slab.
            with tc.tile_pool(name="psY", bufs=2, space="PSUM") as psY, \
                 tc.tile_pool(name="psS", bufs=2, space="PSUM") as psS, \
                 tc.tile_pool(name="pP", bufs=3) as pP, \
                 tc.tile_pool(name="ldram", bufs=2, space="DRAM") as ldram, \
                 tc.tile_pool(name="ytail", bufs=2) as ytail:
                SLAB = min(1024, t)
                NJH = t // SLAB
                for h in range(hpc):
                    ch = h // 2
                    pr = (h % 2) * 64
                    lrow = 64 if h % 2 == 0 else 32
                    for jh in range(NJH):
                        jlo, jhi = jh * SLAB, (jh + 1) * SLAB
                        y_ps = psY.tile([P, SLAB], f32, tag="y", name="y_ps")
                        ic_max = jhi // P
                        for ic in range(ic_max):
                            i0 = ic * P
                            dstart = (i0 // JB) * JB
                            jstart = max(jlo, dstart)
                            w = jhi - jstart
                            nblk = w // JB
                            s_ps = psS.tile([P, SLAB], f32, tag="s", name="s_ps")
                            for bb in range(nblk):
                                jb = jstart + bb * JB
                                nc.tensor.matmul(
                                    s_ps[:, bb * JB:(bb + 1) * JB],
                                    q_sb[:, h, i0:i0 + P],
                                    k_sb[:, ch, jb:jb + JB],
                                    start=True,
                                    stop=True,
                                )
                            p_t = pP.tile([P, SLAB], bf16, tag="p", name="p_t")
                            nc.scalar.activation(
                                p_t[:, :w], s_ps[:, :w], Exp,
                                scale=float(1.0 / np.sqrt(hs)),
                            )
                            if i0 >= jlo:
                                # diagonal block sits at local offset 0
                                m4 = ic % (JB // P)
                                eng = nc.vector if (ic % 2) else nc.gpsimd
                                eng.tensor_tensor(
                                    p_t[:, :JB], p_t[:, :JB], mask_sb[:, m4],
                                    mybir.AluOpType.mult,
                                )
                            for bb in range(nblk):
                                jbg = (jstart + bb * JB) // JB
                                stop_ic = min(ic_max, (jbg + 1) * (JB // P)) - 1
                                nc.tensor.matmul(
                                    y_ps[:, jstart - jlo + bb * JB:
                                         jstart - jlo + (bb + 1) * JB],
                                    v_sb[:, ic, h, :],
                                    p_t[:, bb * JB:(bb + 1) * JB],
                                    start=(ic == 0),
                                    stop=(ic == stop_ic),
                                    skip_group_check=True,
                                )
                        # ---- tail: y / l ----
                        lrow_sb = ytail.tile([P, SLAB], f32, tag="lrow", name="lrow_sb")
                        nc.vector.tensor_copy(
                            lrow_sb[lrow:lrow + 1, :], y_ps[lrow:lrow + 1, :]
                        )
                        l_dram = ldram.tile([SLAB], f32, tag="ld", name="l_dram")
                        nc.sync.dma_start(l_dram[:], lrow_sb[lrow:lrow + 1, :])
                        # partition-parallel reciprocal: [16, 64] layout
                        lpar = ytail.tile([16, SLAB // 16], f32, tag="lpar", name="lpar")
                        nc.sync.dma_start(
                            lpar[:], l_dram.rearrange("(p o) -> p o", p=16)
                        )
                        lpinv = ytail.tile([16, SLAB // 16], f32, tag="lpinv", name="lpinv")
                        nc.vector.reciprocal(lpinv[:], lpar[:])
                        linv_dram = ldram.tile([SLAB], f32, tag="ld2", name="linv_dram")
                        nc.sync.dma_start(
                            linv_dram.rearrange("(p o) -> p o", p=16), lpinv[:]
                        )
                        lb = ytail.tile([P, SLAB], f32, tag="lb", name="lb")
                        nc.sync.dma_start(
                            lb[pr:pr + 64, :], linv_dram.partition_broadcast(64)
                        )
                        nc.vector.tensor_mul(
                            y_sb[pr:pr + 64, ch, jlo:jhi],
                            y_ps[pr:pr + 64, :],
                            lb[pr:pr + 64, :],
                        )
                        if debug_taps:
                            nc.sync.dma_start(dbg["lb"][h][pr:pr + 64, jlo:jhi],
                                              lb[pr:pr + 64, :])

            if debug_taps:
                nc.sync.dma_start(dbg["q"][:], q_sb[:])
                nc.sync.dma_start(dbg["k"][:], k_sb[:])
                nc.sync.dma_start(dbg["v"][:], v_sb[:])
                nc.sync.dma_start(dbg["y"][:], y_sb[:])

            # ---------------- phase C: output projection ----------------
            with tc.tile_pool(name="psC", bufs=4, space="PSUM") as psC, \
                 tc.tile_pool(name="oC", bufs=3) as oC:
                for tt in range(NT):
                    for nh in range(c // 512):
                        ps = psC.tile([P, 512], f32, tag="c", name="psC")
                        for kc in range(dh // P):
                            nc.tensor.matmul(
                                ps[:],
                                y_sb[:, kc, tt * P:(tt + 1) * P],
                                wp_sb[:, kc, nh * 512:(nh + 1) * 512],
                                start=(kc == 0),
                                stop=(kc == dh // P - 1),
                            )
                        o_t = oC.tile([P, 512], f32, tag="o", name="o_t")
                        nc.vector.tensor_copy(o_t[:], ps[:])
                        nc.sync.dma_start(
                            out_d[tt * P:(tt + 1) * P, nh * 512:(nh + 1) * 512],
                            o_t[:],
                        )

    nc.compile()
    return nc


def _get_nc():
    if "nc" not in _NC_CACHE:
        _NC_CACHE["nc"] = build_bass()
    return _NC_CACHE["nc"]


def _head_perm(g, nh=NH, hs=HS, hpc=HPC):
    """Column indices of w_attn's q block for head-group g, ordered (h_local, d)."""
    return np.array(
        [d * nh + (hpc * g + h) for h in range(hpc) for d in range(hs)],
        dtype=np.int64,
    )


def _make_masks():
    """masks[i, m, j] = 1.0 if j >= m*128 + i else 0 ; [128, 4, 512] bf16."""
    i = np.arange(P)[:, None, None]
    m = np.arange(4)[None, :, None]
    j = np.arange(512)[None, None, :]
    return (j >= m * P + i).astype(BF16)


def prepare_in_maps(x, w_attn, b_attn, w_proj):
    """Host-side sharding: returns (in_maps list of 8 dicts, bias correction [C])."""
    x = np.asarray(x, dtype=np.float32)
    w_attn = np.asarray(w_attn, dtype=np.float32)
    b_attn = np.asarray(b_attn, dtype=np.float32)
    w_proj = np.asarray(w_proj, dtype=np.float32)

    masks = _make_masks()
    per_g = {}
    bias_corr = np.zeros((C,), dtype=np.float32)
    for g in (0, 1):
        idx = _head_perm(g)
        wqk = np.concatenate(
            [w_attn[:, idx], w_attn[:, C + idx]], axis=1
        ).astype(BF16)
        wv = np.ascontiguousarray(w_attn[:, 2 * C + idx]).astype(BF16)
        wp = np.ascontiguousarray(w_proj[idx, :]).astype(BF16)
        bk = np.ascontiguousarray(b_attn[C + idx]).astype(np.float32)
        bv = b_attn[2 * C + idx]
        bias_corr += bv.astype(np.float32) @ w_proj[idx, :]
        per_g[g] = (wqk, wv, wp, bk)

    in_maps = []
    for core in range(NCORES):
        b, g = core // 2, core % 2
        wqk, wv, wp, bk = per_g[g]
        in_maps.append(
            {
                "xt": np.ascontiguousarray(x[b].T).astype(BF16),
                "wqk": wqk,
                "wv": wv,
                "wp": wp,
                "bk": bk,
                "masks": masks,
            }
        )
    return in_maps, bias_corr


def combine_outputs(outs, b_proj, bias_corr):
    b_proj = np.asarray(b_proj, dtype=np.float32)
    y = np.empty((B, T, C), dtype=np.float32)
    for b in range(B):
        y[b] = outs[2 * b] + outs[2 * b + 1]
    y += (b_proj + bias_corr)[None, None, :]
    return y


def kernel(x=None, w_attn=None, b_attn=None, w_proj=None, b_proj=None, **_):
    from concourse.bass_utils import run_bass_kernel_spmd

    nc = _get_nc()
    in_maps, bias_corr = prepare_in_maps(x, w_attn, b_attn, w_proj)
    res = run_bass_kernel_spmd(nc, in_maps, core_ids=list(range(NCORES)))
    outs = [np.asarray(res.results[c]["out"], dtype=np.float32) for c in range(NCORES)]
    return combine_outputs(outs, b_proj, bias_corr)


# revision 26
# speedup vs baseline: 1.6666x; 1.0652x over previous
"""Causal self-attention Trainium2 kernel (8 NeuronCores, SPMD, no collectives).

Problem (from reference):
    B=4, T=2048, C=1024, NH=16, HS=64
    qkv = x @ w_attn + b_attn ; heads interleaved: head h uses cols d*NH+h
    att[i,j] = q_i.k_j/sqrt(HS), keep i<=j, softmax over i
    y[d,j] = sum_i v[i,d] p[i,j] ; out = y @ w_proj + b_proj

Sharding: core = 2*b + g  (b in 0..3 batch, g in 0..1 head-group of 8 heads).
Each core computes a partial output (its 8 heads' contribution to out[b]);
host sums the two partials per batch and adds biases.

Device pipeline per core (all bf16 matmuls, fp32 PSUM):
    xT[b] (host pre-transposed, bf16)  --MM-->  qT,kT (channel-major) and
    v (token-major, via lhsT=xT), then per head flash-style causal attention
    with softmax denominator via a ones-column in the V stationary operand,
    then y^T @ w_proj_rows.

Host-side exact bias handling: q-bias drops out of softmax_i (constant in i);
k-bias added on device per-partition; v-bias and b_proj folded in on host.
"""

import os
import sys

import numpy as np

if "/opt/trn_rl_repo" not in sys.path:
    sys.path.insert(0, "/opt/trn_rl_repo")

import ml_dtypes

BF16 = ml_dtypes.bfloat16

B, T, C, NH, HS = 4, 2048, 1024, 16, 64
HPC = 8            # heads per core
DH = HPC * HS      # 512 head-channels per core
P = 128
NCORES = 8

_NC_CACHE = {}


def build_bass(t=T, c=C, hpc=HPC, hs=HS, debug_taps=False):
    """Build + compile the per-core Bass program. Parameterized so small
    versions can be simulated for debugging."""
    import concourse.bass as bass  # noqa: F401
    import concourse.tile as tile
    from concourse import bacc, mybir

    dt = mybir.dt
    f32 = dt.float32
    bf16 = dt.bfloat16
    Exp = mybir.ActivationFunctionType.Exp
    add_op = mybir.AluOpType.add

    dh = hpc * hs
    KC = c // P            # contraction chunks over C
    NT = t // P            # token tiles
    QCH = dh // P          # channel chunks for q (and for k) per core
    NB = t // 512          # 512-wide t blocks
    JB = 512               # j block width

    nc = bacc.Bacc("TRN2", target_bir_lowering=False, debug=False)

    xt_d = nc.dram_tensor("xt", [c, t], bf16, kind="ExternalInput").ap()
    wqk_d = nc.dram_tensor("wqk", [c, 2 * dh], bf16, kind="ExternalInput").ap()
    wv_d = nc.dram_tensor("wv", [c, dh], bf16, kind="ExternalInput").ap()
    wp_d = nc.dram_tensor("wp", [dh, c], bf16, kind="ExternalInput").ap()
    bk_d = nc.dram_tensor("bk", [dh], f32, kind="ExternalInput").ap()
    masks_d = nc.dram_tensor("masks", [P, 4, JB], bf16, kind="ExternalInput").ap()
    ident_d = nc.dram_tensor("ident", [P, P], bf16, kind="ExternalInput").ap()
    out_d = nc.dram_tensor("out", [t, c], f32, kind="ExternalOutput").ap()
    dbg = {}
    if debug_taps:
        dbg["q"] = nc.dram_tensor("dbg_q", [P, hpc, t], bf16, kind="ExternalOutput").ap()
        dbg["k"] = nc.dram_tensor("dbg_k", [P, (hpc * hs) // P, t], bf16, kind="ExternalOutput").ap()
        dbg["v"] = nc.dram_tensor("dbg_v", [P, t // P, hpc, P], bf16, kind="ExternalOutput").ap()
        dbg["y"] = nc.dram_tensor("dbg_y", [P, (hpc * hs) // P, t], bf16, kind="ExternalOutput").ap()
        dbg["lb"] = nc.dram_tensor("dbg_lb", [hpc, P, t], f32, kind="ExternalOutput").ap()
        dbg["yps"] = nc.dram_tensor("dbg_yps", [P, t], f32, kind="ExternalOutput").ap()
        dbg["s0"] = nc.dram_tensor("dbg_s0", [2, P, 1024], f32, kind="ExternalOutput").ap()
        dbg["p0"] = nc.dram_tensor("dbg_p0", [2, P, 1024], bf16, kind="ExternalOutput").ap()

    with tile.TileContext(nc) as tc:
        from contextlib import ExitStack

        with ExitStack() as ctx:
            const = ctx.enter_context(tc.tile_pool(name="const", bufs=1))
            data = ctx.enter_context(tc.tile_pool(name="data", bufs=1))
            # phase-A-only tensors; pool released before attention phase
            adata_cm = tc.tile_pool(name="adata", bufs=1)
            adata = adata_cm.__enter__()

            # ---------------- input loads ----------------
            xt_sb = adata.tile([P, KC, t], bf16)
            xt_r = xt_d.rearrange("(o p) t -> p o t", p=P)
            for kc in range(KC):
                nc.sync.dma_start(xt_sb[:, kc], xt_r[:, kc])

            wqk_sb = adata.tile([P, KC, 2 * dh], bf16)
            wqk_r = wqk_d.rearrange("(o p) n -> p o n", p=P)
            for kc in range(KC):
                nc.sync.dma_start(wqk_sb[:, kc], wqk_r[:, kc])

            wv_sb = adata.tile([P, KC, dh], bf16)
            wv_r = wv_d.rearrange("(o p) n -> p o n", p=P)
            for kc in range(KC):
                nc.sync.dma_start(wv_sb[:, kc], wv_r[:, kc])

            wp_sb = data.tile([P, dh // P, c], bf16)
            wp_r = wp_d.rearrange("(o p) n -> p o n", p=P)
            for kc in range(dh // P):
                nc.sync.dma_start(wp_sb[:, kc], wp_r[:, kc])

            bk_sb = const.tile([P, dh // P], f32)
            nc.sync.dma_start(bk_sb[:], bk_d.rearrange("(o p) -> p o", p=P))

            mask_sb = const.tile([P, 4, JB], bf16)
            nc.sync.dma_start(mask_sb[:], masks_d)
            ident_sb = const.tile([P, P], bf16)
            nc.sync.dma_start(ident_sb[:], ident_d)

            # persistent activations
            # q is stored zero-padded per head: chunk h holds head h's 64
            # channels at partitions pr..pr+63 (pr = (h%2)*64), zeros
            # elsewhere, so S matmuls contract over the full 128 partitions
            # (K<128 matmuls are unreliable on HW).
            q_sb = data.tile([P, hpc, t], bf16)
            k_sb = data.tile([P, QCH, t], bf16)
            nc.gpsimd.memset(q_sb[:], 0.0)
            # V stationary operand: per (token-tile, head) a [P,128] block:
            #   even head: cols 0..63 = v, col 64 = 1.0  -> y at part 0..63, l at 64
            #   odd  head: col 32 = 1.0, cols 64..127 = v -> l at 32, y at part 64..127
            v_sb = data.tile([P, NT, hpc, P], bf16)
            y_sb = data.tile([P, dh // P, t], bf16)

            nc.gpsimd.memset(v_sb[:], 0.0)
            nc.vector.memset(v_sb[:, :, 0::2, 64], 1.0)
            nc.vector.memset(v_sb[:, :, 1::2, 32], 1.0)

            # ---------------- phase A: projections ----------------
            with tc.tile_pool(name="psA", bufs=4, space="PSUM") as psA:
                # q/k channel-major: qkT[cc, t] = sum_c' Wqk[c',cc] * xT[c',t]
                for m in range(2 * QCH):
                    dst = q_sb if m < QCH else k_sb
                    mm = m % QCH
                    for nb in range(NB):
                        ps = psA.tile([P, 512], f32, tag="psA", name="psA")
                        for kc in range(KC):
                            nc.tensor.matmul(
                                ps[:],
                                wqk_sb[:, kc, m * P:(m + 1) * P],
                                xt_sb[:, kc, nb * 512:(nb + 1) * 512],
                                start=(kc == 0),
                                stop=(kc == KC - 1),
                            )
                        if m >= QCH:
                            # fold in k-bias (per output channel == partition)
                            nc.vector.tensor_scalar(
                                dst[:, mm, nb * 512:(nb + 1) * 512],
                                ps[:],
                                bk_sb[:, mm:mm + 1],
                                None,
                                add_op,
                            )
                        else:
                            sl = slice(nb * 512, (nb + 1) * 512)
                            nc.vector.tensor_copy(
                                q_sb[0:64, 2 * mm, sl], ps[0:64]
                            )
                            nc.vector.tensor_copy(
                                q_sb[64:128, 2 * mm + 1, sl], ps[64:128]
                            )
                # v token-major: v[t, dv] = sum_c' xT[c',t] * Wv[c',dv]
                for tt in range(NT):
                    ps = psA.tile([P, dh], f32, tag="psA", name="psA")
                    for kc in range(KC):
                        nc.tensor.matmul(
                            ps[:],
                            xt_sb[:, kc, tt * P:(tt + 1) * P],
                            wv_sb[:, kc, :],
                            start=(kc == 0),
                            stop=(kc == KC - 1),
                        )
                    psh = ps.rearrange("p (h d) -> p h d", d=hs)
                    nc.vector.tensor_copy(v_sb[:, tt, 0::2, 0:hs], psh[:, 0::2])
                    nc.vector.tensor_copy(v_sb[:, tt, 1::2, hs:2 * hs], psh[:, 1::2])

            adata_cm.__exit__(None, None, None)

            # ---------------- phase B: attention ----------------
            # processed per (head, j-slab of 1024): y PSUM double-buffered so
            # the per-slab softmax-denominator tail overlaps the next slab.
            with tc.tile_pool(name="psY", bufs=2, space="PSUM") as psY, \
                 tc.tile_pool(name="psS", bufs=2, space="PSUM") as psS, \
                 tc.tile_pool(name="pP", bufs=3) as pP, \
                 tc.tile_pool(name="ldram", bufs=2, space="DRAM") as ldram, \
                 tc.tile_pool(name="ytail", bufs=2) as ytail:
                SLAB = min(1024, t)
                NJH = t // SLAB
                for h in range(hpc):
                    ch = h // 2
                    pr = (h % 2) * 64
                    lrow = 64 if h % 2 == 0 else 32
                    for jh in range(NJH):
                        jlo, jhi = jh * SLAB, (jh + 1) * SLAB
                        y_ps = psY.tile([P, SLAB], f32, tag="y", name="y_ps")
                        ic_max = jhi // P
                        for ic in range(ic_max):
                            i0 = ic * P
                            dstart = (i0 // JB) * JB
                            jstart = max(jlo, dstart)
                            w = jhi - jstart
                            nblk = w // JB
                            s_ps = psS.tile([P, SLAB], f32, tag="s", name="s_ps")
                            for bb in range(nblk):
                                jb = jstart + bb * JB
                                diag = (bb == 0 and i0 >= jlo)
                                nc.tensor.matmul(
                                    s_ps[:, bb * JB:(bb + 1) * JB],
                                    q_sb[:, h, i0:i0 + P],
                                    k_sb[:, ch, jb:jb + JB],
                                    start=True,
                                    stop=not diag,
                                )
                                if diag:
                                    # add -big upper-left triangle so exp -> 0
                                    nc.tensor.matmul(
                                        s_ps[:, bb * JB:(bb + 1) * JB],
                                        ident_sb[:],
                                        mask_sb[:, ic % (JB // P)],
                                        start=False,
                                        stop=True,
                                    )
                            p_t = pP.tile([P, SLAB], bf16, tag="p", name="p_t")
                            nc.scalar.activation(
                                p_t[:, :w], s_ps[:, :w], Exp,
                                scale=float(1.0 / np.sqrt(hs)),
                            )
                            for bb in range(nblk):
                                jbg = (jstart + bb * JB) // JB
                                stop_ic = min(ic_max, (jbg + 1) * (JB // P)) - 1
                                nc.tensor.matmul(
                                    y_ps[:, jstart - jlo + bb * JB:
                                         jstart - jlo + (bb + 1) * JB],
                                    v_sb[:, ic, h, :],
                                    p_t[:, bb * JB:(bb + 1) * JB],
                                    start=(ic == 0),
                                    stop=(ic == stop_ic),
                                    skip_group_check=True,
                                )
                        # ---- tail: y / l ----
                        lrow_sb = ytail.tile([P, SLAB], f32, tag="lrow", name="lrow_sb")
                        nc.vector.tensor_copy(
                            lrow_sb[lrow:lrow + 1, :], y_ps[lrow:lrow + 1, :]
                        )
                        l_dram = ldram.tile([SLAB], f32, tag="ld", name="l_dram")
                        nc.sync.dma_start(l_dram[:], lrow_sb[lrow:lrow + 1, :])
                        # partition-parallel reciprocal: [16, 64] layout
                        lpar = ytail.tile([16, SLAB // 16], f32, tag="lpar", name="lpar")
                        nc.sync.dma_start(
                            lpar[:], l_dram.rearrange("(p o) -> p o", p=16)
                        )
                        lpinv = ytail.tile([16, SLAB // 16], f32, tag="lpinv", name="lpinv")
                        nc.vector.reciprocal(lpinv[:], lpar[:])
                        linv_dram = ldram.tile([SLAB], f32, tag="ld2", name="linv_dram")
                        nc.sync.dma_start(
                            linv_dram.rearrange("(p o) -> p o", p=16), lpinv[:]
                        )
                        lb = ytail.tile([P, SLAB], f32, tag="lb", name="lb")
                        nc.sync.dma_start(
                            lb[pr:pr + 64, :], linv_dram.partition_broadcast(64)
                        )
                        nc.vector.tensor_mul(
                            y_sb[pr:pr + 64, ch, jlo:jhi],
                            y_ps[pr:pr + 64, :],
                            lb[pr:pr + 64, :],
                        )
                        if debug_taps:
                            nc.sync.dma_start(dbg["lb"][h][pr:pr + 64, jlo:jhi],
                                              lb[pr:pr + 64, :])

            if debug_taps:
                nc.sync.dma_start(dbg["q"][:], q_sb[:])
                nc.sync.dma_start(dbg["k"][:], k_sb[:])
                nc.sync.dma_start(dbg["v"][:], v_sb[:])
                nc.sync.dma_start(dbg["y"][:], y_sb[:])

            # ---------------- phase C: output projection ----------------
            with tc.tile_pool(name="psC", bufs=4, space="PSUM") as psC, \
                 tc.tile_pool(name="oC", bufs=3) as oC:
                for tt in range(NT):
                    for nh in range(c // 512):
                        ps = psC.tile([P, 512], f32, tag="c", name="psC")
                        for kc in range(dh // P):
                            nc.tensor.matmul(
                                ps[:],
                                y_sb[:, kc, tt * P:(tt + 1) * P],
                                wp_sb[:, kc, nh * 512:(nh + 1) * 512],
                                start=(kc == 0),
                                stop=(kc == dh // P - 1),
                            )
                        o_t = oC.tile([P, 512], f32, tag="o", name="o_t")
                        nc.vector.tensor_copy(o_t[:], ps[:])
                        eng = nc.sync if (tt + nh) % 2 == 0 else nc.gpsimd
                        eng.dma_start(
                            out_d[tt * P:(tt + 1) * P, nh * 512:(nh + 1) * 512],
                            o_t[:],
                        )

    nc.compile()
    return nc


def _get_nc():
    if "nc" not in _NC_CACHE:
        _NC_CACHE["nc"] = build_bass()
    return _NC_CACHE["nc"]


def _head_perm(g, nh=NH, hs=HS, hpc=HPC):
    """Column indices of w_attn's q block for head-group g, ordered (h_local, d)."""
    return np.array(
        [d * nh + (hpc * g + h) for h in range(hpc) for d in range(hs)],
        dtype=np.int64,
    )


def _make_masks():
    """Additive mask: 0 where kept (j >= m*128 + i), -60000 where causally
    masked; added to S pre-exp via an identity matmul. [128, 4, 512] bf16."""
    i = np.arange(P)[:, None, None]
    m = np.arange(4)[None, :, None]
    j = np.arange(512)[None, None, :]
    return np.where(j >= m * P + i, 0.0, -60000.0).astype(BF16)


def prepare_in_maps(x, w_attn, b_attn, w_proj):
    """Host-side sharding: returns (in_maps list of 8 dicts, bias correction [C])."""
    x = np.asarray(x, dtype=np.float32)
    w_attn = np.asarray(w_attn, dtype=np.float32)
    b_attn = np.asarray(b_attn, dtype=np.float32)
    w_proj = np.asarray(w_proj, dtype=np.float32)

    masks = _make_masks()
    per_g = {}
    bias_corr = np.zeros((C,), dtype=np.float32)
    for g in (0, 1):
        idx = _head_perm(g)
        wqk = np.concatenate(
            [w_attn[:, idx], w_attn[:, C + idx]], axis=1
        ).astype(BF16)
        wv = np.ascontiguousarray(w_attn[:, 2 * C + idx]).astype(BF16)
        wp = np.ascontiguousarray(w_proj[idx, :]).astype(BF16)
        bk = np.ascontiguousarray(b_attn[C + idx]).astype(np.float32)
        bv = b_attn[2 * C + idx]
        bias_corr += bv.astype(np.float32) @ w_proj[idx, :]
        per_g[g] = (wqk, wv, wp, bk)

    in_maps = []
    for core in range(NCORES):
        b, g = core // 2, core % 2
        wqk, wv, wp, bk = per_g[g]
        in_maps.append(
            {
                "xt": np.ascontiguousarray(x[b].T).astype(BF16),
                "wqk": wqk,
                "wv": wv,
                "wp": wp,
                "bk": bk,
                "masks": masks,
                "ident": np.eye(P, dtype=np.float32).astype(BF16),
            }
        )
    return in_maps, bias_corr


def combine_outputs(outs, b_proj, bias_corr):
    b_proj = np.asarray(b_proj, dtype=np.float32)
    y = np.empty((B, T, C), dtype=np.float32)
    for b in range(B):
        y[b] = outs[2 * b] + outs[2 * b + 1]
    y += (b_proj + bias_corr)[None, None, :]
    return y


def kernel(x=None, w_attn=None, b_attn=None, w_proj=None, b_proj=None, **_):
    from concourse.bass_utils import run_bass_kernel_spmd

    nc = _get_nc()
    in_maps, bias_corr = prepare_in_maps(x, w_attn, b_attn, w_proj)
    res = run_bass_kernel_spmd(nc, in_maps, core_ids=list(range(NCORES)))
    outs = [np.asarray(res.results[c]["out"], dtype=np.float32) for c in range(NCORES)]
    return combine_outputs(outs, b_proj, bias_corr)
